# revision 58
# baseline (speedup 1.0000x reference)
"""Trainium2 Bass kernel for nn_BCCLayer (bilinear co-attention + pooling + batchnorm).

Math
----
The reference computes, per batch b, two bilinear attention maps
G = (relu(P@Wq^T+Qb)*h_mat) @ relu(R@Wk^T+Kb)^T  of shape [2000, 2000],
applies a masked softmax over the first (v) axis, contracts with the
K-side features, mean-pools over the sequence, and batchnorms over the
batch. Because the softmax mask depends only on the column index and the
softmax normalizes over rows, the per-element attention weights are never
needed - only two column sums of exp(G):

  S_all[q] = sum_v exp(G[v,q])
  S_w[q]   = sum_v mask_p[v] * exp(G[v,q])
  w[q]     = mask_v[q]/L * S_w[q]/S_all[q]
  contrib[k] = sum_q w[q] * V[q,k]

(any per-column shift of G - including h_bias - cancels in the ratio,
and |G| < ~1 so exp needs no max-subtraction).

h_mat enters G as sum_k U[v,k] (h[k]*V[q,k]): 64*h (signed) is folded
into the V-side fp8 weights and bias on the host, so
h*relu(z) = max(pm+b, 0) for h>0 tiles and min(pm+b, 0) for h<0 tiles -
a single fused evacuation either way. The k channels are permuted
(positives first) so the sign is uniform per 128-partition tile; the one
straddling tile (if npos % 128 != 0) ships |h| weights and gets a Pool
sign-multiply after its relu (partition slices must be 32-aligned).

Engine budget (per core, one (batch, direction) unit):
- ACT: exp over the valid-q window (the dominant unavoidable cost,
  ~28us) plus a few startup FC evacuations while otherwise idle.
- PE: fp8 DoubleRow matmuls for the feature FCs / G / S-reduction
  (cost-model: 0.5 cycles/row), f32r value chain, tiny S transposes.
- DVE: all psum evacuations (GPSIMD cannot read PSUM), w math.
- Pool: SBUF-only work (f32r rounding copies, sign fixups, rbuf).
All input transposes are done on the host; inputs ship pre-packed.

Schedule: the q window is processed in spans of <=8 128-col tiles
(PSUM: 2x2-bank G psums + 1 packed S bank + 2 FC banks). Per span the
16 v-tile-pairs stream G->exp->S-reduction; everything else (remaining
feature FCs, the value-chain FCs, the previous span's w math + contrib)
is interleaved as filler at hand-tuned slots so the exp stream never
starves. S chunks accumulate in ONE psum bank at 32-aligned partition
offsets. Span contributions DMA out separately; the host sums them and
runs the [4,512] batchnorm epilogue.

Sharding: 8 independent (batch, map) units -> one per NeuronCore, SPMD.
fp8 scaling: W x64 (its ~3e-3 entries would be subnormal in e4m3); the
exp applies the 1/64^2 correction via the ACT affine. Only q columns
with mask_v > 0 contribute, so the host permutes valid columns to the
front and the window shrinks to ceil(max_valid/128) tiles, chosen at
runtime from the actual masks.
"""

import numpy as np

L = 2000
LP = 2048  # L padded to a multiple of 128
HD = 256
KD = 512
B = 4
EPS = 1e-5
NCORES = 8
WSCALE = 64.0   # fp8 weight scale
NLT = LP // 128  # 16 v tiles

_NC_CACHE = {}


def _build_nc(nqt, npos):
    """nqt: q window in 128-col tiles. npos: number of k channels with
    h >= 0 after the sign permutation (boundary of the max/min evac)."""
    import concourse.mybir as mybir
    import concourse.tile as tile
    from concourse import bacc

    f32 = mybir.dt.float32
    bf16 = mybir.dt.bfloat16
    fp8 = mybir.dt.float8e4
    f32r = mybir.dt.float32r
    AF = mybir.ActivationFunctionType
    ALU = mybir.AluOpType
    DR = mybir.MatmulPerfMode.DoubleRow

    nc = bacc.Bacc("TRN2", target_bir_lowering=False)

    NQ = nqt * 128          # packed q window (valid cols first)
    # q tiles processed in spans of <=8 tiles (psum limit: 2 s-banks + 2x2
    # gp banks + 2 fc banks = 8)
    spans = []
    t0 = 0
    while t0 < nqt:
        w = min(8, nqt - t0)
        spans.append((t0, w))
        t0 += w
    span0w = spans[0][1] * 128

    # host-pretransposed inputs (see kernel() for layouts)
    p8t_in = nc.dram_tensor("p8t_in", [128, 2, LP], fp8, kind="ExternalInput")
    r8t_in = nc.dram_tensor("r8t_in", [128, 2, NQ], fp8, kind="ExternalInput")
    rt_in = nc.dram_tensor("rt_in", [128, 2, NQ], f32, kind="ExternalInput")
    w8pack_in = nc.dram_tensor("w8pack_in", [128, 2, 2 * KD], fp8, kind="ExternalInput")
    wkr_in = nc.dram_tensor("wkr_in", [128, 2, KD], f32, kind="ExternalInput")
    # f32 pack: cols 0-127 identity; 128-139 bias (64*Qb, 64*h*Kb, unused as
    # 4-col tiles); 140+: mask_p {0,1} x16, valid {0,1} x16, mask_v/L x nqt
    f32small_in = nc.dram_tensor(
        "f32small_in", [128, 140 + 32 + nqt], f32, kind="ExternalInput"
    )
    kb_div = nc.dram_tensor("kb_div", [KD], f32, kind="ExternalInput")  # Kb/128
    out = nc.dram_tensor("out", [2, KD], f32, kind="ExternalOutput")

    NKC = KD // 128   # 4 k chunks

    with tile.TileContext(nc) as tc:
        import contextlib
        ctx = contextlib.ExitStack()
        with ctx:
            singles = ctx.enter_context(tc.tile_pool(name="singles", bufs=1))
            wsmall = ctx.enter_context(tc.tile_pool(name="wsmall", bufs=16))
            epool = ctx.enter_context(tc.tile_pool(name="epool", bufs=3))
            pfc = ctx.enter_context(tc.tile_pool(name="pfc", bufs=2, space="PSUM"))
            pg = ctx.enter_context(tc.tile_pool(name="pg", bufs=2, space="PSUM"))
            ps = ctx.enter_context(tc.tile_pool(name="ps", bufs=2, space="PSUM"))

            # ---- input DMAs: one queue, in need-order (the DMA engine pool
            # is a serial resource; transfer order == issue order) ----
            # gate-first DMA order: the first 512 q/v columns unblock the
            # first G chunk long before the full tensors land
            NQg = min(512, NQ)
            r8t = singles.tile([128, 2, NQ], fp8)
            nc.sync.dma_start(r8t[:, :, 0:NQg], r8t_in[:, :, 0:NQg])
            w8 = singles.tile([128, 2, 2 * KD], fp8)
            nc.sync.dma_start(w8, w8pack_in[:])
            f32s = singles.tile([128, 140 + 32 + nqt], f32)
            nc.sync.dma_start(f32s, f32small_in[:])
            p8t = singles.tile([128, 2, LP], fp8)
            nc.sync.dma_start(p8t[:, :, 0:512], p8t_in[:, :, 0:512])
            if NQ > NQg:
                nc.sync.dma_start(r8t[:, :, NQg:NQ], r8t_in[:, :, NQg:NQ])
            nc.sync.dma_start(p8t[:, :, 512:LP], p8t_in[:, :, 512:LP])
            rt_sb = singles.tile([128, 2, NQ], f32)
            nc.sync.dma_start(rt_sb, rt_in[:])
            wk_st = singles.tile([128, 2, KD], f32)
            nc.sync.dma_start(wk_st, wkr_in[:])
            kbd_st = singles.tile([128, KD], f32)

            wq8 = w8[:, :, 0:KD]
            wk8 = w8[:, :, KD : 2 * KD]
            ident = f32s[:, 0:128]
            bcols = f32s[:, 128:140]
            mcols = f32s[:, 140:]

            # warm the PE clock from t=0 until the first DMA-gated matmul
            ones_st = singles.tile([128, 128], f32)
            nc.vector.memset(ones_st, 1.0)
            ones_t = singles.tile([128, 128], f32r)
            nc.gpsimd.tensor_copy(ones_t, ones_st)
            warm_ps = pfc.tile([128, 512], f32, tag="fc")
            for _ in range(14):
                nc.tensor.transpose(warm_ps[:, 0:128], ones_st, ones_st)
            # prime the ACT exp table while ACT is idle
            warm_act = wsmall.tile([1, 1], f32, tag="wa")
            nc.scalar.activation(warm_act, ones_st[0:1, 0:1], AF.Exp)

            qb64_col = bcols[:, 0:NKC]               # 64*Qb
            kb64_col = bcols[:, NKC : 2 * NKC]       # 64*h*Kb
            sgn_col = bcols[:, 2 * NKC : 3 * NKC]    # sign(h), straddle tile
            mp_col = mcols[:, 0:NLT]          # numerator mask, {0,1}
            valid_col = mcols[:, NLT : 2 * NLT]
            mv_col = mcols[:, 2 * NLT :]      # output mask, {0,1/L}, packed

            # reduction stationary, DoubleRow-paired over v-tile pairs:
            # rbuf8[p, ko, ltp, m]: ko = which v-tile of the pair, m = [valid, mask_p]
            rbuf8 = singles.tile([128, 2, NLT // 2, 2], fp8)

            def emit_rbuf():
                for ko in range(2):
                    nc.gpsimd.tensor_copy(rbuf8[:, ko, :, 0], valid_col[:, ko::2])
                    nc.gpsimd.tensor_copy(rbuf8[:, ko, :, 1], mp_col[:, ko::2])

            # f32r operands must come from a rounding engine op (the BIR
            # verifier rejects bitcast views of DMA'd f32); Pool does the
            # one-time conversions while otherwise idle
            wk_sb = singles.tile([128, 2, KD], f32r)
            nc.gpsimd.tensor_copy(wk_sb, wk_st)
            kbd_bc = singles.tile([128, KD], f32r)
            rt_r = singles.tile([128, 2, NQ], f32r)
            nc.gpsimd.tensor_copy(rt_r, rt_sb)

            # ---- FC-T plumbing. Only span-0 V chunks + U chunk 0 are
            # emitted before the G loop (split across ACT and DVE while ACT
            # is idle); everything else is interleaved into the G loop as
            # filler so the Activation engine streams exps with no gaps. ----
            vt8 = singles.tile([128, NKC, NQ], fp8)
            ut8 = singles.tile([128, NKC, LP], fp8)
            vmid = ctx.enter_context(tc.tile_pool(name="vmid", bufs=2))

            _pre_rr = [0]

            def fc_pool(kc, pre):
                # pre-loop FC rotates psums across all three pools (pg and ps
                # are idle until the G loop starts) so the PE never stalls on
                # evacs; in-loop fillers must stay off the pg/ps slots.
                if pre:
                    _pre_rr[0] += 1
                    pool = (pfc, pg, ps)[_pre_rr[0] % 3]
                    return pool.tile(
                        [128, 512], f32, tag=("fc", "g", "s")[_pre_rr[0] % 3],
                        name="fcpm",
                    )
                return pfc.tile([128, 512], f32, tag="fc", name="fcpm0")

            def v_kind(kc):
                """"max" / "min" fused-sign relu, or "mix" for the one k tile
                that straddles the h-sign boundary (it ships |h| weights and
                gets a Pool sign multiply after the relu)."""
                a, b = kc * 128, (kc + 1) * 128
                if b <= npos:
                    return "max"
                if a >= npos:
                    return "min"
                return "mix"

            def v_evac(kc, sl, cw, pm, eng):
                # eng: "A" ACT (max/mix pieces only), "D" DVE. GPSIMD cannot
                # read PSUM, so Pool only does the post-relu sign multiply.
                kind = v_kind(kc)
                dst = vt8[:, kc, sl]
                if kind == "mix":
                    dst = vmid.tile([128, 512], bf16, tag="vm", name="vm")[:, :cw]
                if eng == "A" and kind in ("max", "mix"):
                    nc.scalar.activation(
                        dst, pm[:, :cw], AF.Relu,
                        bias=kb64_col[:, kc : kc + 1],
                    )
                else:
                    nc.vector.tensor_scalar(
                        dst, pm[:, :cw],
                        kb64_col[:, kc : kc + 1], 0.0, ALU.add,
                        ALU.min if kind == "min" else ALU.max,
                    )
                if kind == "mix":
                    nc.gpsimd.tensor_scalar_mul(
                        vt8[:, kc, sl], dst, sgn_col[:, kc : kc + 1]
                    )

            def u_evac(kc, sl, pm, eng):
                if eng == "A":
                    nc.scalar.activation(
                        ut8[:, kc, sl], pm, AF.Relu,
                        bias=qb64_col[:, kc : kc + 1],
                    )
                else:
                    nc.vector.tensor_scalar(
                        ut8[:, kc, sl], pm,
                        qb64_col[:, kc : kc + 1], 0.0, ALU.add, ALU.max,
                    )

            def fc_v(c0, cw, pre=False, engs="DPDP"):
                sl = slice(c0, c0 + cw)
                for kc in range(NKC):
                    pm = fc_pool(kc, pre)
                    nc.tensor.matmul(
                        pm[:, :cw],
                        lhsT=wk8[:, :, kc * 128 : (kc + 1) * 128],
                        rhs=r8t[:, :, sl],
                        perf_mode=DR,
                    )
                    v_evac(kc, sl, cw, pm, engs[kc])

            def fc_u(vc, pre=False, engs="DPDP"):
                sl = slice(vc * 512, (vc + 1) * 512)
                for kc in range(NKC):
                    pm = fc_pool(kc, pre)
                    nc.tensor.matmul(
                        pm,
                        lhsT=wq8[:, :, kc * 128 : (kc + 1) * 128],
                        rhs=p8t[:, :, sl],
                        perf_mode=DR,
                    )
                    u_evac(kc, sl, pm, engs[kc])

            # value chain: vnat = relu(R@Wk^T + Kb), fp32, Pool evac
            vnat = singles.tile([128, nqt, KD], f32r)

            def fc_nat(qt):
                pm = pfc.tile([128, 512], f32, tag="fc")
                for hc in range(2):
                    nc.tensor.matmul(
                        pm,
                        lhsT=rt_r[:, hc, qt * 128 : (qt + 1) * 128],
                        rhs=wk_sb[:, hc, :],
                        start=(hc == 0),
                        stop=False,
                    )
                nc.tensor.matmul(
                    pm, lhsT=ones_t, rhs=kbd_bc[:],
                    start=False, stop=True, skip_group_check=True,
                )
                nc.vector.tensor_scalar_max(vnat[:, qt, :], pm, 0.0)

            # ---- w = mask_v/L * S_w/S_all + per-span contrib ----
            wcol = singles.tile([128, nqt], f32r)
            s_sb = singles.tile([2, NQ], f32)
            out_sbs = [
                singles.tile([1, KD], f32, name=f"out_sb{i}")
                for i in range(len(spans))
            ]
            s_ps_live = {}
            c_ps_live = {}

            def s_flush(si):
                # S psum -> SBUF right when the last reduction lands, so the
                # next span can reuse the psum slots without aliasing
                st0, snt = spans[si]
                q0 = st0 * 128
                width = snt * 128
                c = 0
                ci = 0
                while c < width:
                    cw = min(512, width - c)
                    nc.vector.tensor_copy(
                        s_sb[:, q0 + c : q0 + c + cw], s_ps_live[si][ci]
                    )
                    c += cw
                    ci += 1

            def span_epilogue(si):
                st0, snt = spans[si]
                q0 = st0 * 128
                # all transposes into one psum tile, then 3 strided DVE ops
                st = pfc.tile([128, 512], f32, tag="fc")
                for j in range(snt):
                    qt = st0 + j
                    nc.tensor.transpose(
                        st[:, 2 * j : 2 * j + 2],
                        s_sb[:, qt * 128 : (qt + 1) * 128], ident[:2, :2],
                    )
                rcp = wsmall.tile([128, 8], f32, tag="rcp")
                nc.vector.reciprocal(rcp[:, :snt], st[:, 0 : 2 * snt : 2])
                nc.vector.tensor_mul(rcp[:, :snt], rcp[:, :snt], st[:, 1 : 2 * snt : 2])
                nc.vector.tensor_mul(
                    wcol[:, st0 : st0 + snt], rcp[:, :snt],
                    mv_col[:, st0 : st0 + snt],
                )
            def span_contrib(si, qts=None):
                st0, snt = spans[si]
                all_qts = list(range(st0, st0 + snt))
                qts = all_qts if qts is None else qts
                # per-span contrib accumulated into psum, DMA'd out per span
                # (the host sums the span rows)
                c_ps = c_ps_live.setdefault(
                    si, pfc.tile([128, 512], f32, tag="fc", name=f"c_ps{si}")
                )
                for qt in qts:
                    nc.tensor.matmul(
                        c_ps[0:1, :],
                        lhsT=wcol[:, qt : qt + 1],
                        rhs=vnat[:, qt, :],
                        start=(qt == all_qts[0]),
                        stop=(qt == all_qts[-1]),
                    )
                if qts[-1] == all_qts[-1]:
                    ob = out_sbs[si]
                    nc.scalar.copy(ob, c_ps[0:1, :])
                    if si == len(spans) - 1:
                        nc.sync.dma_start(out[si : si + 1, :], ob)
                    else:
                        # earlier rows go out via the idle SWDGE queue so the
                        # final row's DMA is not stuck behind them on SP
                        nc.gpsimd.dma_start(out[si : si + 1, :], ob)

            # filler work fed into the G loop: u/v feature chunks go early
            # (paced by PE-cost budget); value-chain FCs go strictly one per
            # v-tile pair so their evacs never pile up on the pfc slots
            uvq = []   # (pe_cost_ns, thunk) - one (chunk, kc) piece each
            engcyc = "DDDD"
            for vc in range(1, LP // 512):
                for kc in range(NKC):
                    def up(vc=vc, kc=kc):
                        sl = slice(vc * 512, (vc + 1) * 512)
                        pm = fc_pool(kc, False)
                        nc.tensor.matmul(
                            pm,
                            lhsT=wq8[:, :, kc * 128 : (kc + 1) * 128],
                            rhs=p8t[:, :, sl],
                            perf_mode=DR,
                        )
                        u_evac(kc, sl, pm, engcyc[kc])
                    uvq.append((160, up))
            c = span0w
            while c < NQ:
                cw = min(512, NQ - c)
                for kc in range(NKC):
                    def vp(c=c, cw=cw, kc=kc):
                        sl = slice(c, c + cw)
                        pm = fc_pool(kc, False)
                        nc.tensor.matmul(
                            pm[:, :cw],
                            lhsT=wk8[:, :, kc * 128 : (kc + 1) * 128],
                            rhs=r8t[:, :, sl],
                            perf_mode=DR,
                        )
                        v_evac(kc, sl, cw, pm, engcyc[kc])
                    uvq.append((cw * 0.31, vp))
                c += 512
            natq = list(range(nqt))

            def pop_filler(budget, nat_ok=True, nnat=1):
                for _ in range(nnat):
                    if nat_ok and natq:
                        fc_nat(natq.pop(0))
                while uvq and budget > 0:
                    cost, thunk = uvq.pop(0)
                    thunk()
                    budget -= cost

            def flush_fillers():
                while uvq:
                    uvq.pop(0)[1]()
                while natq:
                    fc_nat(natq.pop(0))

            # span-0 V features + U chunk 0 gate the first exp: ladder the
            # evac pieces across ACT/DVE/Pool so no single engine serializes
            fc_v(0, NQg, pre=True, engs="AADD")
            fc_u(0, pre=True, engs="DAAD")
            nc.gpsimd.dma_start(kbd_st, kb_div[:].partition_broadcast(128))
            nc.gpsimd.tensor_copy(kbd_bc, kbd_st)
            c = 512
            while c < span0w:
                fc_v(c, min(512, span0w - c), pre=True, engs="ADAD")
                c += 512
            emit_rbuf()
            late_gate = []

            # ---- G (fp8 DR) + exp + fp8 DR column-sum reduction ----
            for si, (st0, snt) in enumerate(spans):
                width = snt * 128
                q0 = st0 * 128
                chunks = []
                c = 0
                while c < width:
                    chunks.append((c, min(512, width - c)))
                    c += 512
                # ACT time per v-pair in this span (2 exps) minus G+reduction
                slack = 2 * (width * 0.8333 + 185) - (width * 2.5) * 0.5 * 0.4167
                for ltp in range(NLT // 2):    # pairs of v tiles
                    et = epool.tile([128, 2, width], fp8, tag="e")
                    for sub in range(2):
                        lt = 2 * ltp + sub
                        if si == 0 and ltp == 0:
                            # chunk-wise G+exp with per-chunk psum tiles on
                            # the very first v pair: dep tracking is per pool
                            # tile, so ACT can start on chunk 0 before the
                            # later chunks' inputs even exist
                            for c0, cw in chunks:
                                gpc = pg.tile([128, 512], f32, tag="g", name="gpc")
                                for j in range(2):
                                    nc.tensor.matmul(
                                        gpc[:, :cw],
                                        lhsT=ut8[:, 2 * j : 2 * j + 2, lt * 128 : (lt + 1) * 128],
                                        rhs=vt8[:, 2 * j : 2 * j + 2, q0 + c0 : q0 + c0 + cw],
                                        start=(j == 0),
                                        stop=(j == 1),
                                        perf_mode=DR,
                                    )
                                nc.scalar.activation(
                                    et[:, sub, c0 : c0 + cw],
                                    gpc[:, :cw], AF.Exp,
                                    scale=1.0 / (WSCALE * WSCALE),
                                )
                                if sub == 0 and c0 == 0:
                                    while late_gate:
                                        lc, lw = late_gate.pop(0)
                                        fc_v(lc, lw, pre=True, engs="ADAD")
                            continue
                        gp = pg.tile([128, width], f32, tag="g")
                        for c0, cw in chunks:
                            for j in range(2):
                                nc.tensor.matmul(
                                    gp[:, c0 : c0 + cw],
                                    lhsT=ut8[:, 2 * j : 2 * j + 2, lt * 128 : (lt + 1) * 128],
                                    rhs=vt8[:, 2 * j : 2 * j + 2, q0 + c0 : q0 + c0 + cw],
                                    start=(j == 0),
                                    stop=(j == 1),
                                    perf_mode=DR,
                                )
                        nc.scalar.activation(
                            et[:, sub, :], gp[:, :width], AF.Exp,
                            scale=1.0 / (WSCALE * WSCALE),
                        )
                    if ltp == 0:
                        # allocated here (not at span start) so the pre-gate
                        # psum rotation through the ps slots stays ordered
                        s_ps_live[si] = [
                            ps.tile([2, cw], f32, tag="s", name=f"s_ps_{si}_{ci}")
                            for ci, (c0, cw) in enumerate(chunks)
                        ]
                    for ci, (c0, cw) in enumerate(chunks):
                        nc.tensor.matmul(
                            s_ps_live[si][ci],
                            lhsT=rbuf8[:, :, ltp, :],
                            rhs=et[:, :, c0 : c0 + cw],
                            start=(ltp == 0), stop=(ltp == NLT // 2 - 1),
                            perf_mode=DR,
                            skip_group_check=True,
                        )
                    st0p, sntp = spans[si - 1] if si else (0, 0)
                    if si > 0 and ltp == 0:
                        # previous span's epilogue runs under this span's
                        # first exps instead of stalling the PE at the seam
                        span_epilogue(si - 1)
                    elif si > 0 and ltp == 3:
                        span_contrib(si - 1, list(range(st0p, st0p + sntp))[: sntp // 2])
                    elif si > 0 and ltp == 4:
                        span_contrib(si - 1, list(range(st0p, st0p + sntp))[sntp // 2 :])
                    else:
                        nnat = 0
                        if si == 0 and ltp >= 4:
                            nnat = 1
                        elif si > 0 and ltp == 1:
                            nnat = 3
                        elif si > 0 and ltp == 2:
                            nnat = 1
                        elif si > 0 and ltp in (5, 6):
                            nnat = 2
                        elif si > 0:
                            nnat = 1
                        pop_filler(slack - 400, nnat=nnat)
                s_flush(si)
                if si == len(spans) - 1:
                    flush_fillers()
                    span_epilogue(si)
                    span_contrib(si)

    nc.finalize()
    return nc


def _get_nc(nqt=13, npos=256):
    key = (nqt, npos)
    if key not in _NC_CACHE:
        _NC_CACHE[key] = _build_nc(nqt, npos)
    return _NC_CACHE[key]


def kernel(**inputs) -> np.ndarray:
    import ml_dtypes
    from concourse.bass_utils import run_bass_kernel_spmd

    X = np.asarray(inputs["X"], dtype=np.float32)
    Y = np.asarray(inputs["Y"], dtype=np.float32)
    m1 = np.asarray(inputs["mask1"], dtype=np.float32)
    m2 = np.asarray(inputs["mask2"], dtype=np.float32)
    Qv = np.asarray(inputs["Qv"], dtype=np.float32)
    Qg = np.float32(np.asarray(inputs["Qg"]))
    Qb = np.asarray(inputs["Qb"], dtype=np.float32)
    Kv = np.asarray(inputs["Kv"], dtype=np.float32)
    Kg = np.float32(np.asarray(inputs["Kg"]))
    Kb = np.asarray(inputs["Kb"], dtype=np.float32)
    hm = np.asarray(inputs["h_mat"], dtype=np.float32)
    gamma = np.asarray(inputs["gamma"], dtype=np.float32)
    beta = np.asarray(inputs["beta"], dtype=np.float32)

    f8 = ml_dtypes.float8_e4m3

    Wq = (Qg / np.float32(np.linalg.norm(Qv))) * Qv  # [KD, HD]
    Wk = (Kg / np.float32(np.linalg.norm(Kv))) * Kv

    # permute k channels so h>=0 ones come first; fold 64*h into the V-side
    # fp8 weights/bias (sign handled by the max/min evac split at npos)
    kperm = np.argsort(hm < 0, kind="stable")
    npos = int((hm >= 0).sum())
    Wq_p = Wq[kperm]
    Qb_p = Qb[kperm]
    hm_p = hm[kperm]
    hfold = hm_p.copy()
    sgn_p = np.ones(KD, np.float32)
    if npos % 128 != 0:
        # the k tile straddling the sign boundary ships |h| weights and a
        # Pool-engine sign multiply (partition slices must be 32-aligned)
        t = npos // 128
        blk = slice(t * 128, (t + 1) * 128)
        hfold[blk] = np.abs(hm_p[blk])
        sgn_p[blk] = np.sign(hm_p[blk])
        sgn_p[blk][hm_p[blk] == 0] = 1.0
    Wkh_p = hfold[:, None] * Wk[kperm]
    Kbh_p = hfold * Kb[kperm]

    # 64*W^T packed [128, 2, KD]: arr[p, c, k] = 64*W[k, 2p+c]
    wq8 = np.ascontiguousarray((WSCALE * Wq_p.T).reshape(128, 2, KD).astype(f8))
    wk8 = np.ascontiguousarray((WSCALE * Wkh_p.T).reshape(128, 2, KD).astype(f8))
    w8pack_in = np.concatenate([wq8, wk8], axis=2)
    # Wk^T [128, 2, KD]: arr[p, hc, k] = Wk[k, hc*128+p]  (unpermuted, f32)
    wkr_in = np.ascontiguousarray(
        Wk.T.reshape(2, 128, KD).transpose(1, 0, 2)
    ).astype(np.float32)

    bias_cols = np.concatenate(
        [(WSCALE * Qb_p).reshape(4, 128), (WSCALE * Kbh_p).reshape(4, 128),
         sgn_p.reshape(4, 128)], axis=0
    ).T.astype(np.float32)  # [128, 12]
    kb_div = (Kb / 128.0).astype(np.float32)
    ident = np.eye(128, dtype=np.float32)

    def padded(v2000):
        p = np.zeros((LP,), np.float32)
        p[:L] = v2000
        return p.reshape(16, 128)

    valid = padded(np.ones(L, np.float32))

    # Only q columns with mask_v > 0 contribute; permute them to the front
    # and size the computed q window (in 128-col tiles) to cover every valid
    # column across all 8 cores.
    units = []
    max_nv = 0
    for b in range(B):
        for m in range(2):
            if m == 0:
                P, R, mp, mv = X[b], Y[b], m1[b], m2[b]
            else:
                P, R, mp, mv = Y[b], X[b], m2[b], m1[b]
            perm = np.argsort(mv <= 0, kind="stable")
            max_nv = max(max_nv, int((mv > 0).sum()))
            units.append((P, R, mp, mv, perm))
    nqt = min(16, max(1, -(-max_nv // 128)))
    NQ = 128 * nqt

    in_maps = []
    for P, R, mp, mv, perm in units:
        nperm = min(NQ, L)
        Rp = np.zeros((NQ, HD), np.float32)
        Rp[:nperm] = R[perm[:nperm]]
        mvp = np.zeros((NQ,), np.float32)
        mvp[:nperm] = mv[perm[:nperm]] * (1.0 / L)
        f32small_in = np.ascontiguousarray(
            np.concatenate(
                [ident, bias_cols,
                 np.concatenate(
                     [padded(mp), valid, mvp.reshape(nqt, 128)], axis=0
                 ).T],
                axis=1,
            )
        ).astype(np.float32)  # [128, 140 + 32 + nqt]
        Ppad = np.zeros((LP, HD), np.float32)
        Ppad[:L] = P
        # [128, 2, n] with row (p, c) = h channel 2p+c
        p8t = np.ascontiguousarray(Ppad.T.astype(f8).reshape(128, 2, LP))
        r8t = np.ascontiguousarray(Rp.T.astype(f8).reshape(128, 2, NQ))
        # R^T bf16 [128, 2, NQ]: arr[p, hc, q] = R[q, hc*128+p]
        rt = np.ascontiguousarray(
            Rp.T.reshape(2, 128, NQ).transpose(1, 0, 2)
        ).astype(np.float32)
        in_maps.append(
            {
                "p8t_in": p8t,
                "r8t_in": r8t,
                "rt_in": rt,
                "w8pack_in": w8pack_in,
                "wkr_in": wkr_in,
                "f32small_in": f32small_in,
                "kb_div": kb_div,
            }
        )

    nc = _get_nc(nqt, npos)
    res = run_bass_kernel_spmd(nc, in_maps, core_ids=list(range(NCORES)))
    nspans = -(-nqt // 8)
    contribs = np.stack(
        [r["out"][:nspans].sum(axis=0) for r in res.results]
    ).astype(np.float64)

    pooled = contribs[0::2] + contribs[1::2]  # [B, KD]
    mu = pooled.mean(axis=0)
    var = pooled.var(axis=0)
    outv = gamma * (pooled - mu) / np.sqrt(var + EPS) + beta
    return outv.astype(np.float32)
